# revision 27
# baseline (speedup 1.0000x reference)
"""Causal self-attention (GPT-style, 12 heads, C=768) on 8 TRN2 NeuronCores.

Sharding: core c -> (batch b = c//2, head-group g = c%2 of 6 heads).
Each core computes qkv projection for its 6 heads, causal attention, and a
partial output projection (its 384 rows of w_proj). Host sums the two
partial projections per batch (row-parallel tensor parallelism); b_proj is
folded into the g=0 core's partial.

v2 pipeline design (HAM-warm scheduling):
  - score psum groups are per-j-tile [128, 2(heads), 512] tiles (2 banks),
    ring of 2 -> depth-2 software pipeline: scores(g+1) issue while exp(g)
    runs on ScalarE, AV(g) follows. One exp op covers both heads.
  - AV accumulates into one [65, 2, 512] psum (M=65: V plus a ones column
    so row 64 accumulates the softmax denominator S).
  - softmax normalization is deferred: av rows 0..63 are copied to outT
    unnormalized; S rows are gathered (partitions 0/32/64 of an SBUF
    table) and 1/S = exp(-ln S) is computed once per chunk in one batched
    Ln + Exp pair. Per (pair, head): K=1 broadcast matmul of 1/S then one
    in-place DVE multiply on outT.
  - independent matmul work (next chunk's qkv projection, previous chunk's
    normalize broadcasts + output projection) is interleaved between
    attention groups as PE filler so the tensor engine never idles long
    enough for the HAM clock gate to re-throttle it to 1.2 GHz.
  - single shared 2-slot PSUM scratch ring serves qkv/proj/broadcast;
    8 PSUM banks total: 4 (scores) + 2 (AV) + 2 (scratch).
"""

import numpy as np

import concourse.bass as bass
import concourse.mybir as mybir
import concourse.tile as tile
from concourse import bacc
from concourse import bass_utils

f32 = mybir.dt.float32
bf16 = mybir.dt.bfloat16
AF = mybir.ActivationFunctionType
ALU = mybir.AluOpType

N_HEAD = 12
N_EMBD = 768
B_FULL = 4
T_FULL = 2048
N_CORES = 8
SCALE = float(N_EMBD) ** -0.5

TRACE = False
LAST_RESULT = None
_NC_CACHE = {}


def build_nc(T=T_FULL, dbg=False):
    C = N_EMBD            # 768
    NP = 3                # head pairs (6 local heads)
    KT = C // 128         # 6 k-tiles for the projections
    NIC = T // 512        # i-chunks (512 queries each)
    NJT = T // 128        # j-tiles (128 keys each)

    # Pin Exp/Ln to the one activation-table set containing both, so the
    # table-load pass emits a single load.
    import concourse.bacc as _bacc_mod
    from concourse.hw_specs import get_activation_tables as _orig_gat

    def _pinned_gat(arch):
        tabs = {k: set(v) for k, v in _orig_gat(arch).items()}
        for name, fns in tabs.items():
            if name != "natural_log_exp_and_others":
                fns.discard(AF.Exp)
                fns.discard(AF.Ln)
        return tabs

    nc = bacc.Bacc("TRN2", target_bir_lowering=False, debug=False)

    xT_d = nc.dram_tensor("xT", [C, T], bf16, kind="ExternalInput")
    wqk_d = nc.dram_tensor("wqk", [C, 768], bf16, kind="ExternalInput")
    wv_d = nc.dram_tensor("wv", [C, 384], bf16, kind="ExternalInput")
    wp_d = nc.dram_tensor("wp", [384, C], bf16, kind="ExternalInput")
    bqk_d = nc.dram_tensor("bqk", [128, 6], f32, kind="ExternalInput")
    bv_d = nc.dram_tensor("bv", [1, 384], bf16, kind="ExternalInput")
    bp_d = nc.dram_tensor("bp", [1, C], bf16, kind="ExternalInput")
    ones_d = nc.dram_tensor("ones", [128, 128], bf16, kind="ExternalInput")
    mask_d = nc.dram_tensor("mask", [128, 2, 128], bf16, kind="ExternalInput")
    y_d = nc.dram_tensor("y", [T, C], f32, kind="ExternalOutput")

    with tile.TileContext(nc) as tc:
        with (
            tc.tile_pool(name="const", bufs=1) as constp,
            tc.tile_pool(name="xt", bufs=1) as xtp,
            tc.tile_pool(name="qk", bufs=1) as qkp,
            tc.tile_pool(name="vs", bufs=16) as vsp,
            tc.tile_pool(name="es", bufs=6) as esp,
            tc.tile_pool(name="ot", bufs=1) as otp,
            tc.tile_pool(name="sn", bufs=1) as snp,
            tc.tile_pool(name="ys", bufs=4) as ysp,
            tc.tile_pool(name="psg", bufs=2, space="PSUM") as psgp,
            tc.tile_pool(name="pav", bufs=1, space="PSUM") as pavp,
            tc.tile_pool(name="psc", bufs=2, space="PSUM") as pscp,
        ):
            # ---------------- startup constants + prefetch ----------------
            # HAM warm-up: dummy matmuls on a memset tile (no DMA dep) trip
            # the PE clock gate to 8/8 before the first real matmuls arrive.
            warm_src = constp.tile([128, 128], bf16, tag="wsrc")
            nc.vector.memset(warm_src[:], 0.5)
            warm = psgp.tile([128, 2, 512], f32, tag="sg", name="warm")
            for i in range(140):
                nc.tensor.matmul(warm[:, i % 2, 0:128], warm_src[:],
                                 warm_src[:], start=True, stop=True)

            # queue plan: the first qkv matmuls need wqk[k=0..] (sync head)
            # and xt0[k=0..] (gpsimd head); everything else queues behind.
            wqk_t = constp.tile([128, KT, 768], bf16, tag="wqk")
            wqk_src = wqk_d.ap().rearrange("(k p) c -> p k c", p=128)
            nc.sync.dma_start(wqk_t[:, 0:2, :], wqk_src[:, 0:2, :])
            nc.scalar.dma_start(wqk_t[:, 2:4, :], wqk_src[:, 2:4, :])
            wqk = [wqk_t[:, k, :] for k in range(KT)]

            xts_tiles = [xtp.tile([128, KT, 512], bf16, tag=f"xt{ic}",
                                  name=f"xt{ic}") for ic in range(NIC)]
            xsrc = xT_d.ap().rearrange("(k p) t -> p k t", p=128)

            def xdma(eng, ic, k0, k1):
                eng.dma_start(xts_tiles[ic][:, k0:k1, :],
                              xsrc[:, k0:k1, 512 * ic:512 * (ic + 1)])

            xdma(nc.gpsimd, 0, 0, 3)
            nc.gpsimd.dma_start(wqk_t[:, 4:6, :], wqk_src[:, 4:6, :])
            xdma(nc.sync, 0, 3, 6)
            wv_t = constp.tile([128, KT, 384], bf16, tag="wv")
            wv_src = wv_d.ap().rearrange("(k p) c -> p k c", p=128)
            nc.scalar.dma_start(wv_t[:], wv_src)
            wv = [wv_t[:, k, :] for k in range(KT)]
            ones = constp.tile([128, 128], bf16, tag="ones")
            nc.sync.dma_start(ones[:], ones_d.ap()[:])
            bv_row = constp.tile([1, 384], bf16, tag="bvr")
            nc.sync.dma_start(bv_row[:], bv_d.ap()[:])
            bqk_t = constp.tile([128, 6], f32, tag="bqk")
            nc.sync.dma_start(bqk_t[:], bqk_d.ap()[:])
            bqk = [bqk_t[:, m:m + 1] for m in range(6)]

            def gate_xdma(ic, engs):
                # WAW-gate: the tiny memset on the (busy) DVE queue delays
                # the DMA issue so prefetches don't steal SDMA bandwidth
                # from the startup-critical wqk/xt0 transfers.
                def emit():
                    nc.vector.memset(xts_tiles[ic][0:1, 0:1, 0:1], 0.0)
                    xdma(engs[0], ic, 0, 3)
                    xdma(engs[1], ic, 3, 6)
                return emit

            # softmax-denominator table: rows at partitions 32p hold, per
            # chunk, [2 heads x 512] sums; memset so Ln of unused rows is
            # well-defined.
            S_all = snp.tile([65, NIC, 2, 512], f32, tag="sall")
            nc.gpsimd.memset(S_all[:], 1.0)
            rr_all = snp.tile([65, NIC, 2, 512], bf16, tag="rrall")

            # v tiles: [keys 128, local head, 64 v-dims + ones column]
            v = [vsp.tile([128, 6, 65], bf16, tag="v", name=f"v{j}")
                 for j in range(NJT)]
            for j in range(NJT):
                nc.vector.memset(v[j][:, :, 64:65], 1.0)

            # bvb: bias row broadcast to 128 partitions via K=1 matmul
            bvb = constp.tile([128, 384], f32, tag="bvb")
            ps0 = pscp.tile([128, 512], f32, tag="sc", name="ps_bvb")
            nc.tensor.matmul(ps0[:, 0:384], ones[0:1, :], bv_row[:],
                             start=True, stop=True)
            nc.vector.tensor_copy(bvb[:], ps0[:, 0:384])
            bvb_r = bvb[:].rearrange("p (h d) -> p h d", h=6)

            # ---------------- qkv projection units ------------------------
            qT = [qkp.tile([128, T], bf16, tag=f"qT{p}", name=f"qT{p}")
                  for p in range(NP)]
            kT = [qkp.tile([128, T], bf16, tag=f"kT{p}", name=f"kT{p}")
                  for p in range(NP)]

            def qk_unit(tci, m):
                def emit():
                    ps = pscp.tile([128, 512], f32, tag="sc",
                                   name=f"psqk{tci}_{m}")
                    xts = xts_tiles[tci]
                    for k in range(KT):
                        nc.tensor.matmul(ps[:],
                                         wqk[k][:, 128 * m:128 * (m + 1)],
                                         xts[:, k, :],
                                         start=(k == 0), stop=(k == KT - 1))
                    dest = qT[m] if m < 3 else kT[m - 3]
                    nc.vector.tensor_scalar_add(
                        dest[:, 512 * tci:512 * (tci + 1)], ps[:], bqk[m])
                return emit

            def v_unit(tci, tsub):
                def emit():
                    ps = pscp.tile([128, 512], f32, tag="sc",
                                   name=f"psv{tci}_{tsub}")
                    xts = xts_tiles[tci]
                    jt = 4 * tci + tsub
                    for k in range(KT):
                        nc.tensor.matmul(
                            ps[:, 0:384],
                            xts[:, k, 128 * tsub:128 * (tsub + 1)],
                            wv[k], start=(k == 0), stop=(k == KT - 1))
                    nc.vector.tensor_tensor(
                        v[jt][:, :, 0:64],
                        ps[:, 0:384].rearrange("p (h d) -> p h d", h=6),
                        bvb_r, op=ALU.add)
                return emit

            def qkv_units(tci):
                us = [qk_unit(tci, m) for m in range(6)]
                us += [v_unit(tci, tsub) for tsub in range(4)]
                return us

            def q_units(tci):
                return [qk_unit(tci, m) for m in range(3)]

            def kv_units(tci):
                return ([qk_unit(tci, m) for m in range(3, 6)]
                        + [v_unit(tci, tsub) for tsub in range(4)])

            # ---------------- late consts (proj weights, mask) ------------
            outT = [otp.tile([128, T], bf16, tag=f"outT{p}", name=f"outT{p}")
                    for p in range(NP)]
            late = {}

            def emit_late_consts():
                msk = constp.tile([128, 2, 128], bf16, tag="msk")
                nc.sync.dma_start(msk[:], mask_d.ap()[:])
                wp_t = constp.tile([128, NP, 768], bf16, tag="wp")
                nc.scalar.dma_start(
                    wp_t[:], wp_d.ap().rearrange("(m p) c -> p m c", p=128))
                bp_row = constp.tile([1, 768], bf16, tag="bpr")
                nc.sync.dma_start(bp_row[:], bp_d.ap()[:])
                bpb = constp.tile([128, 768], f32, tag="bpb")
                for lo, hi in [(0, 512), (512, 768)]:
                    ps = pscp.tile([128, 512], f32, tag="sc", name="ps_bpb")
                    nc.tensor.matmul(ps[:, 0:hi - lo], ones[0:1, :],
                                     bp_row[:, lo:hi], start=True, stop=True)
                    nc.vector.tensor_copy(bpb[:, lo:hi], ps[:, 0:hi - lo])
                late["msk"] = msk
                late["wp"] = [wp_t[:, m, :] for m in range(NP)]
                late["bpb"] = bpb

            # ---------------- normalize + projection units ----------------
            def norm_unit(ic, p, h):
                def emit():
                    dsl = slice(64 * h, 64 * (h + 1))
                    isl = slice(512 * ic, 512 * (ic + 1))
                    sl = pscp.tile([128, 512], f32, tag="sc",
                                   name=f"rbp{ic}_{p}_{h}")
                    rbp = sl[0:64, :]
                    nc.tensor.matmul(rbp,
                                     ones[32 * p:32 * p + 1, 0:64],
                                     rr_all[32 * p:32 * p + 1, ic, h, :],
                                     start=True, stop=True)
                    nc.vector.tensor_tensor(outT[p][dsl, isl],
                                            outT[p][dsl, isl], rbp,
                                            op=ALU.mult)
                return emit

            def proj_unit(ic, tsub, pool=None):
                def emit():
                    wp, bpb = late["wp"], late["bpb"]
                    t0 = 512 * ic + 128 * tsub
                    ysb = ysp.tile([128, 768], f32, tag="y",
                                   name=f"y{ic}_{tsub}")
                    for n in range(2):
                        nsl = slice(384 * n, 384 * (n + 1))
                        pl = pool if pool is not None else pscp
                        sl = pl.tile([128, 512], f32,
                                     tag="sc" if pl is pscp else "sg",
                                     name=f"yp{ic}_{tsub}_{n}")
                        yp = sl[:, 0:384]
                        for mp in range(NP):
                            nc.tensor.matmul(
                                yp, outT[mp][:, t0:t0 + 128], wp[mp][:, nsl],
                                start=(mp == 0), stop=(mp == NP - 1))
                        nc.vector.tensor_tensor(ysb[:, nsl], yp,
                                                bpb[:, nsl], op=ALU.add)
                        eng = nc.sync if n == 0 else nc.scalar
                        eng.dma_start(y_d.ap()[t0:t0 + 128, nsl],
                                      ysb[:, nsl])
                return emit

            def norm_proj_units(ic):
                us = [norm_unit(ic, p, h) for p in range(NP) for h in (0, 1)]
                us += [proj_unit(ic, tsub) for tsub in range(4)]
                return us

            # ---------------- attention pairs -----------------------------
            pairs = [(0, slice(0, 64)), (1, slice(64, 128))]

            class Filler:
                def __init__(self):
                    self.units = []
                    self.credit = 0.0
                    self.quota = 0.0

                def add(self, us, groups_left):
                    self.units.extend(us)
                    self.quota = len(self.units) / max(groups_left, 1)

                def step(self):
                    self.credit += self.quota
                    while self.units and self.credit >= 1.0:
                        self.units.pop(0)()
                        self.credit -= 1.0

                def flush(self):
                    for u in self.units:
                        u()
                    self.units = []

            def emit_scores(ic, p, g):
                isl = slice(512 * ic, 512 * (ic + 1))
                kind = g[0]
                if kind in ("full", "d0"):
                    jt = g[1] if kind == "full" else 4 * ic
                    sg = psgp.tile([128, 2, 512], f32, tag="sg",
                                   name=f"sg{ic}_{p}_{kind}{jt}")
                    for h, dsl in pairs:
                        nc.tensor.matmul(
                            sg[:, h, :],
                            kT[p][dsl, 128 * jt:128 * (jt + 1)],
                            qT[p][dsl, isl], start=True, stop=True)
                    return (sg, 512)
                if kind == "d1":
                    jt = 4 * ic + 1
                    sg = psgp.tile([128, 2, 512], f32, tag="sg",
                                   name=f"sgd1_{ic}_{p}")
                    for h, dsl in pairs:
                        nc.tensor.matmul(
                            sg[:, h, 0:384],
                            kT[p][dsl, 128 * jt:128 * (jt + 1)],
                            qT[p][dsl, 512 * ic + 128:512 * ic + 512],
                            start=True, stop=True)
                    return (sg, 384)
                # d23: two decreasing blocks packed per head
                sg = psgp.tile([128, 2, 512], f32, tag="sg",
                               name=f"sgd23_{ic}_{p}")
                for h, dsl in pairs:
                    nc.tensor.matmul(
                        sg[:, h, 0:256],
                        kT[p][dsl, 128 * (4 * ic + 2):128 * (4 * ic + 3)],
                        qT[p][dsl, 512 * ic + 256:512 * ic + 512],
                        start=True, stop=False)
                    nc.tensor.matmul(
                        sg[:, h, 256:384],
                        kT[p][dsl, 128 * (4 * ic + 3):128 * (4 * ic + 4)],
                        qT[p][dsl, 512 * ic + 384:512 * ic + 512],
                        start=False, stop=True)
                return (sg, 384)

            def emit_exp_av(ic, p, g, sg_w, avt, first):
                # causal mask is additive (-30000 above the diagonal),
                # applied to the score psum BEFORE exp so the DVE stays off
                # the exp -> AV critical path.
                msk = late["msk"]
                kind = g[0]
                sg, w = sg_w
                if kind in ("d0", "d1", "d23"):
                    nc.vector.tensor_tensor(sg[:, :, 0:128],
                                            sg[:, :, 0:128], msk[:],
                                            op=ALU.add)
                if kind == "d23":
                    nc.vector.tensor_tensor(sg[:, :, 256:384],
                                            sg[:, :, 256:384], msk[:],
                                            op=ALU.add)
                et = esp.tile([128, 2, w], bf16, tag="et",
                              name=f"et{ic}_{p}_{kind}")
                nc.scalar.activation(et[:], sg[:, :, 0:w], AF.Exp,
                                     scale=SCALE)
                for h, dsl in pairs:
                    hl = 2 * p + h
                    if kind == "full":
                        nc.tensor.matmul(avt[:, h, :], v[g[1]][:, hl, :],
                                         et[:, h, :],
                                         start=first, stop=False)
                    elif kind == "d0":
                        nc.tensor.matmul(avt[:, h, :],
                                         v[4 * ic][:, hl, :], et[:, h, :],
                                         start=first, stop=False)
                    elif kind == "d1":
                        nc.tensor.matmul(avt[:, h, 128:512],
                                         v[4 * ic + 1][:, hl, :],
                                         et[:, h, :],
                                         start=False, stop=False)
                    else:
                        nc.tensor.matmul(avt[:, h, 256:512],
                                         v[4 * ic + 2][:, hl, :],
                                         et[:, h, 0:256],
                                         start=False, stop=False)
                        nc.tensor.matmul(avt[:, h, 384:512],
                                         v[4 * ic + 3][:, hl, :],
                                         et[:, h, 256:384],
                                         start=False, stop=True)

            def emit_lnexp(ic, rows=slice(0, 65)):
                # rr = exp(-ln S): batched 1/S over the S rows of a chunk
                lnS = snp.tile([65, 2, 512], f32, tag="lnS", bufs=2,
                               name=f"lnS{ic}_{rows.start}")
                n = rows.stop - rows.start
                nc.scalar.activation(lnS[0:n], S_all[rows, ic], AF.Ln)
                nc.scalar.activation(rr_all[rows, ic], lnS[0:n],
                                     AF.Exp, scale=-1.0)

            def emit_attn_pair(ic, p, fill, inline_norm=False):
                isl = slice(512 * ic, 512 * (ic + 1))
                avt = pavp.tile([65, 2, 512], f32, tag="av",
                                name=f"av{ic}_{p}")
                glist = [("full", jt) for jt in range(4 * ic)]
                glist += [("d0",), ("d1",), ("d23",)]
                sg_w = emit_scores(ic, p, glist[0])
                for i, g in enumerate(glist):
                    cur = sg_w
                    if i + 1 < len(glist):
                        sg_w = emit_scores(ic, p, glist[i + 1])
                    fill.step()
                    emit_exp_av(ic, p, g, cur, avt, first=(i == 0))
                # evacuate: unnormalized outT + S rows
                for h, dsl in pairs:
                    nc.vector.tensor_copy(outT[p][dsl, isl],
                                          avt[0:64, h, :])
                nc.vector.tensor_copy(S_all[32 * p:32 * p + 1, ic],
                                      avt[64:65, :, :])
                if inline_norm:
                    emit_lnexp(ic, rows=slice(32 * p, 32 * p + 1))
                    for h in (0, 1):
                        norm_unit(ic, p, h)()

            # ---------------- main schedule -------------------------------
            us0 = qkv_units(0)
            us0[0]()
            gate_xdma(1, (nc.gpsimd, nc.sync))()
            for u in us0[1:4]:
                u()
            gate_xdma(2, (nc.scalar, nc.gpsimd))()
            for u in us0[4:7]:
                u()
            gate_xdma(3, (nc.sync, nc.scalar))()
            for u in us0[7:]:
                u()
            emit_late_consts()
            fill = Filler()

            fill.add(qkv_units(1), 9)
            for p in range(NP):
                emit_attn_pair(0, p, fill)
            emit_lnexp(0)

            fill.add(qkv_units(2), 21)
            for p in range(NP):
                emit_attn_pair(1, p, fill)
            emit_lnexp(1)

            fill.add(qkv_units(3) + norm_proj_units(0), 33)
            for p in range(NP):
                emit_attn_pair(2, p, fill)
            emit_lnexp(2)

            fill.add(norm_proj_units(1) + norm_proj_units(2), 45)
            for p in range(NP):
                emit_attn_pair(3, p, fill, inline_norm=True)
            fill.flush()
            for tsub in range(4):
                proj_unit(3, tsub, pool=psgp if tsub % 2 else None)()

    _bacc_mod.get_activation_tables = _pinned_gat
    try:
        nc.compile()
    finally:
        _bacc_mod.get_activation_tables = _orig_gat
    return nc


def make_in_maps(x, w_attn, b_attn, w_proj, b_proj, T=T_FULL):
    import ml_dtypes
    bf = ml_dtypes.bfloat16
    x = np.asarray(x, np.float32)
    w_attn = np.asarray(w_attn, np.float32)
    b_attn = np.asarray(b_attn, np.float32)
    w_proj = np.asarray(w_proj, np.float32)
    b_proj = np.asarray(b_proj, np.float32)
    B = x.shape[0]

    ones = np.ones((128, 128), bf)
    # additive causal mask for the leading 128-column diagonal sub-block of
    # each stripe matmul: 0 on/below the diagonal, -30000 above (added to
    # the raw scores before exp; two identical copies, one per head)
    mask = np.broadcast_to(
        (np.arange(128)[:, None, None] > np.arange(128)[None, None, :]),
        (128, 2, 128)).astype(np.float32) * -30000.0

    in_maps = []
    for c in range(N_CORES):
        b, g = (c // 2) % B, c % 2
        q0, k0, v0 = 384 * g, 768 + 384 * g, 1536 + 384 * g
        wqk = np.concatenate(
            [w_attn[:, q0:q0 + 384], w_attn[:, k0:k0 + 384]], axis=1)
        bqk = np.concatenate(
            [b_attn[q0:q0 + 384], b_attn[k0:k0 + 384]])
        in_maps.append({
            "xT": np.ascontiguousarray(x[b].T).astype(bf),
            "wqk": np.ascontiguousarray(wqk).astype(bf),
            "wv": np.ascontiguousarray(w_attn[:, v0:v0 + 384]).astype(bf),
            "wp": np.ascontiguousarray(w_proj[384 * g:384 * (g + 1), :]).astype(bf),
            "bqk": np.ascontiguousarray(bqk.reshape(6, 128).T),
            "bv": np.ascontiguousarray(b_attn[v0:v0 + 384].reshape(1, 384)).astype(bf),
            "bp": np.ascontiguousarray(
                (b_proj if g == 0 else np.zeros_like(b_proj)).reshape(1, -1)).astype(bf),
            "ones": ones,
            "mask": np.ascontiguousarray(mask).astype(bf),
        })
    return in_maps


def kernel(x, w_attn, b_attn, w_proj, b_proj):
    global LAST_RESULT
    if "nc" not in _NC_CACHE:
        _NC_CACHE["nc"] = build_nc(T_FULL)
    nc = _NC_CACHE["nc"]
    in_maps = make_in_maps(x, w_attn, b_attn, w_proj, b_proj)
    res = bass_utils.run_bass_kernel_spmd(
        nc, in_maps, core_ids=list(range(N_CORES)), trace=TRACE)
    LAST_RESULT = res
    B, T, C = np.asarray(x).shape
    y = np.empty((B, T, C), np.float32)
    for b in range(B):
        y[b] = res.results[2 * b]["y"] + res.results[2 * b + 1]["y"]
    return y


# revision 29
# speedup vs baseline: 1.0785x; 1.0785x over previous
"""Causal self-attention (GPT-style, 12 heads, C=768) on 8 TRN2 NeuronCores.

Sharding: core c -> (batch b = c//2, head-group g = c%2 of 6 heads).
Each core computes qkv projection for its 6 heads, causal attention, and a
partial output projection (its 384 rows of w_proj). Host sums the two
partial projections per batch (row-parallel tensor parallelism); b_proj is
folded into the g=0 core's partial.

v2 pipeline design (HAM-warm scheduling):
  - score psum groups are per-j-tile [128, 2(heads), 512] tiles (2 banks),
    ring of 2 -> depth-2 software pipeline: scores(g+1) issue while exp(g)
    runs on ScalarE, AV(g) follows. One exp op covers both heads.
  - AV accumulates into one [65, 2, 512] psum (M=65: V plus a ones column
    so row 64 accumulates the softmax denominator S).
  - softmax normalization is deferred: av rows 0..63 are copied to outT
    unnormalized; S rows are gathered (partitions 0/32/64 of an SBUF
    table) and 1/S = exp(-ln S) is computed once per chunk in one batched
    Ln + Exp pair. Per (pair, head): K=1 broadcast matmul of 1/S then one
    in-place DVE multiply on outT.
  - independent matmul work (next chunk's qkv projection, previous chunk's
    normalize broadcasts + output projection) is interleaved between
    attention groups as PE filler so the tensor engine never idles long
    enough for the HAM clock gate to re-throttle it to 1.2 GHz.
  - single shared 2-slot PSUM scratch ring serves qkv/proj/broadcast;
    8 PSUM banks total: 4 (scores) + 2 (AV) + 2 (scratch).
"""

import numpy as np

import concourse.bass as bass
import concourse.mybir as mybir
import concourse.tile as tile
from concourse import bacc
from concourse import bass_utils

f32 = mybir.dt.float32
bf16 = mybir.dt.bfloat16
AF = mybir.ActivationFunctionType
ALU = mybir.AluOpType

N_HEAD = 12
N_EMBD = 768
B_FULL = 4
T_FULL = 2048
N_CORES = 8
SCALE = float(N_EMBD) ** -0.5

TRACE = False
LAST_RESULT = None
_NC_CACHE = {}


def build_nc(T=T_FULL, dbg=False):
    C = N_EMBD            # 768
    NP = 3                # head pairs (6 local heads)
    KT = C // 128         # 6 k-tiles for the projections
    NIC = T // 512        # i-chunks (512 queries each)
    NJT = T // 128        # j-tiles (128 keys each)

    # Pin Exp/Ln to the one activation-table set containing both, so the
    # table-load pass emits a single load.
    import concourse.bacc as _bacc_mod
    from concourse.hw_specs import get_activation_tables as _orig_gat

    def _pinned_gat(arch):
        tabs = {k: set(v) for k, v in _orig_gat(arch).items()}
        for name, fns in tabs.items():
            if name != "natural_log_exp_and_others":
                fns.discard(AF.Exp)
                fns.discard(AF.Ln)
        return tabs

    nc = bacc.Bacc("TRN2", target_bir_lowering=False, debug=False)

    xT_d = nc.dram_tensor("xT", [C, T], bf16, kind="ExternalInput")
    wqk_d = nc.dram_tensor("wqk", [C, 768], bf16, kind="ExternalInput")
    wv_d = nc.dram_tensor("wv", [C, 384], bf16, kind="ExternalInput")
    wp_d = nc.dram_tensor("wp", [384, C], bf16, kind="ExternalInput")
    bqk_d = nc.dram_tensor("bqk", [128, 6], f32, kind="ExternalInput")
    bv_d = nc.dram_tensor("bv", [1, 384], bf16, kind="ExternalInput")
    bp_d = nc.dram_tensor("bp", [1, C], bf16, kind="ExternalInput")
    ones_d = nc.dram_tensor("ones", [128, 128], bf16, kind="ExternalInput")
    mask_d = nc.dram_tensor("mask", [128, 2, 128], bf16, kind="ExternalInput")
    y_d = nc.dram_tensor("y", [T, C], f32, kind="ExternalOutput")

    with tile.TileContext(nc) as tc:
        with (
            tc.tile_pool(name="const", bufs=1) as constp,
            tc.tile_pool(name="xt", bufs=1) as xtp,
            tc.tile_pool(name="qk", bufs=1) as qkp,
            tc.tile_pool(name="vs", bufs=16) as vsp,
            tc.tile_pool(name="es", bufs=6) as esp,
            tc.tile_pool(name="ot", bufs=1) as otp,
            tc.tile_pool(name="sn", bufs=1) as snp,
            tc.tile_pool(name="ys", bufs=4) as ysp,
            tc.tile_pool(name="psg", bufs=2, space="PSUM") as psgp,
            tc.tile_pool(name="pav", bufs=1, space="PSUM") as pavp,
            tc.tile_pool(name="psc", bufs=2, space="PSUM") as pscp,
        ):
            # ---------------- startup constants + prefetch ----------------
            # HAM warm-up: dummy matmuls on a memset tile (no DMA dep) trip
            # the PE clock gate to 8/8 before the first real matmuls arrive.
            warm_src = constp.tile([128, 128], bf16, tag="wsrc")
            nc.vector.memset(warm_src[:], 0.5)
            warm = psgp.tile([128, 2, 512], f32, tag="sg", name="warm")
            for i in range(140):
                nc.tensor.matmul(warm[:, i % 2, 0:128], warm_src[:],
                                 warm_src[:], start=True, stop=True)

            # queue plan: the first qkv matmuls need wqk[k=0..] (sync head)
            # and xt0[k=0..] (gpsimd head); everything else queues behind.
            wqk_t = constp.tile([128, KT, 768], bf16, tag="wqk")
            wqk_src = wqk_d.ap().rearrange("(k p) c -> p k c", p=128)
            nc.sync.dma_start(wqk_t[:, 0:2, :], wqk_src[:, 0:2, :])
            nc.scalar.dma_start(wqk_t[:, 2:4, :], wqk_src[:, 2:4, :])
            wqk = [wqk_t[:, k, :] for k in range(KT)]

            xts_tiles = [xtp.tile([128, KT, 512], bf16, tag=f"xt{ic}",
                                  name=f"xt{ic}") for ic in range(NIC)]
            xsrc = xT_d.ap().rearrange("(k p) t -> p k t", p=128)

            def xdma(eng, ic, k0, k1):
                eng.dma_start(xts_tiles[ic][:, k0:k1, :],
                              xsrc[:, k0:k1, 512 * ic:512 * (ic + 1)])

            xdma(nc.gpsimd, 0, 0, 3)
            nc.gpsimd.dma_start(wqk_t[:, 4:6, :], wqk_src[:, 4:6, :])
            xdma(nc.sync, 0, 3, 6)
            wv_t = constp.tile([128, KT, 384], bf16, tag="wv")
            wv_src = wv_d.ap().rearrange("(k p) c -> p k c", p=128)
            nc.scalar.dma_start(wv_t[:], wv_src)
            wv = [wv_t[:, k, :] for k in range(KT)]
            ones = constp.tile([128, 128], bf16, tag="ones")
            nc.sync.dma_start(ones[:], ones_d.ap()[:])
            bv_row = constp.tile([1, 384], bf16, tag="bvr")
            nc.sync.dma_start(bv_row[:], bv_d.ap()[:])
            bqk_t = constp.tile([128, 6], f32, tag="bqk")
            nc.sync.dma_start(bqk_t[:], bqk_d.ap()[:])
            bqk = [bqk_t[:, m:m + 1] for m in range(6)]

            def gate_xdma(ic, engs):
                # WAW-gate: the tiny memset on the (busy) DVE queue delays
                # the DMA issue so prefetches don't steal SDMA bandwidth
                # from the startup-critical wqk/xt0 transfers.
                def emit():
                    nc.vector.memset(xts_tiles[ic][0:1, 0:1, 0:1], 0.0)
                    xdma(engs[0], ic, 0, 3)
                    xdma(engs[1], ic, 3, 6)
                return emit

            # softmax-denominator table: rows at partitions 32p hold, per
            # chunk, [2 heads x 512] sums; memset so Ln of unused rows is
            # well-defined.
            S_all = snp.tile([65, NIC, 2, 512], f32, tag="sall")
            nc.gpsimd.memset(S_all[:], 1.0)
            rr_all = snp.tile([65, NIC, 2, 512], bf16, tag="rrall")

            # v tiles: [keys 128, local head, 64 v-dims + ones column]
            v = [vsp.tile([128, 6, 65], bf16, tag="v", name=f"v{j}")
                 for j in range(NJT)]
            for j in range(NJT):
                nc.vector.memset(v[j][:, :, 64:65], 1.0)

            # bvb: bias row broadcast to 128 partitions via K=1 matmul
            bvb = constp.tile([128, 384], f32, tag="bvb")
            ps0 = pscp.tile([128, 512], f32, tag="sc", name="ps_bvb")
            nc.tensor.matmul(ps0[:, 0:384], ones[0:1, :], bv_row[:],
                             start=True, stop=True)
            nc.vector.tensor_copy(bvb[:], ps0[:, 0:384])
            bvb_r = bvb[:].rearrange("p (h d) -> p h d", h=6)

            # ---------------- qkv projection units ------------------------
            qT = [qkp.tile([128, T], bf16, tag=f"qT{p}", name=f"qT{p}")
                  for p in range(NP)]
            kT = [qkp.tile([128, T], bf16, tag=f"kT{p}", name=f"kT{p}")
                  for p in range(NP)]

            def qk_unit(tci, m):
                def emit():
                    ps = pscp.tile([128, 512], f32, tag="sc",
                                   name=f"psqk{tci}_{m}")
                    xts = xts_tiles[tci]
                    for k in range(KT):
                        nc.tensor.matmul(ps[:],
                                         wqk[k][:, 128 * m:128 * (m + 1)],
                                         xts[:, k, :],
                                         start=(k == 0), stop=(k == KT - 1))
                    dest = qT[m] if m < 3 else kT[m - 3]
                    nc.vector.tensor_scalar_add(
                        dest[:, 512 * tci:512 * (tci + 1)], ps[:], bqk[m])
                return emit

            def v_unit(tci, tsub):
                def emit():
                    ps = pscp.tile([128, 512], f32, tag="sc",
                                   name=f"psv{tci}_{tsub}")
                    xts = xts_tiles[tci]
                    jt = 4 * tci + tsub
                    for k in range(KT):
                        nc.tensor.matmul(
                            ps[:, 0:384],
                            xts[:, k, 128 * tsub:128 * (tsub + 1)],
                            wv[k], start=(k == 0), stop=(k == KT - 1))
                    nc.vector.tensor_tensor(
                        v[jt][:, :, 0:64],
                        ps[:, 0:384].rearrange("p (h d) -> p h d", h=6),
                        bvb_r, op=ALU.add)
                return emit

            def qkv_units(tci):
                us = [qk_unit(tci, m) for m in range(6)]
                us += [v_unit(tci, tsub) for tsub in range(4)]
                return us

            def q_units(tci):
                return [qk_unit(tci, m) for m in range(3)]

            def kv_units(tci):
                return ([qk_unit(tci, m) for m in range(3, 6)]
                        + [v_unit(tci, tsub) for tsub in range(4)])

            # ---------------- late consts (proj weights, mask) ------------
            outT = [otp.tile([128, T], bf16, tag=f"outT{p}", name=f"outT{p}")
                    for p in range(NP)]
            late = {}

            def emit_late_consts():
                msk = constp.tile([128, 2, 128], bf16, tag="msk")
                nc.sync.dma_start(msk[:], mask_d.ap()[:])
                wp_t = constp.tile([128, NP, 768], bf16, tag="wp")
                nc.scalar.dma_start(
                    wp_t[:], wp_d.ap().rearrange("(m p) c -> p m c", p=128))
                bp_row = constp.tile([1, 768], bf16, tag="bpr")
                nc.sync.dma_start(bp_row[:], bp_d.ap()[:])
                bpb = constp.tile([128, 768], f32, tag="bpb")
                for lo, hi in [(0, 512), (512, 768)]:
                    ps = pscp.tile([128, 512], f32, tag="sc", name="ps_bpb")
                    nc.tensor.matmul(ps[:, 0:hi - lo], ones[0:1, :],
                                     bp_row[:, lo:hi], start=True, stop=True)
                    nc.vector.tensor_copy(bpb[:, lo:hi], ps[:, 0:hi - lo])
                late["msk"] = msk
                late["wp"] = [wp_t[:, m, :] for m in range(NP)]
                late["bpb"] = bpb

            # ---------------- normalize + projection units ----------------
            def norm_unit(ic, p, h):
                def emit():
                    dsl = slice(64 * h, 64 * (h + 1))
                    isl = slice(512 * ic, 512 * (ic + 1))
                    sl = pscp.tile([128, 512], f32, tag="sc",
                                   name=f"rbp{ic}_{p}_{h}")
                    rbp = sl[0:64, :]
                    nc.tensor.matmul(rbp,
                                     ones[32 * p:32 * p + 1, 0:64],
                                     rr_all[32 * p:32 * p + 1, ic, h, :],
                                     start=True, stop=True)
                    nc.vector.tensor_tensor(outT[p][dsl, isl],
                                            outT[p][dsl, isl], rbp,
                                            op=ALU.mult)
                return emit

            def proj_unit(ic, tsub, pool=None):
                def emit():
                    wp, bpb = late["wp"], late["bpb"]
                    t0 = 512 * ic + 128 * tsub
                    ysb = ysp.tile([128, 768], f32, tag="y",
                                   name=f"y{ic}_{tsub}")
                    for n in range(2):
                        nsl = slice(384 * n, 384 * (n + 1))
                        pl = pool if pool is not None else pscp
                        sl = pl.tile([128, 512], f32,
                                     tag="sc" if pl is pscp else "sg",
                                     name=f"yp{ic}_{tsub}_{n}")
                        yp = sl[:, 0:384]
                        for mp in range(NP):
                            nc.tensor.matmul(
                                yp, outT[mp][:, t0:t0 + 128], wp[mp][:, nsl],
                                start=(mp == 0), stop=(mp == NP - 1))
                        nc.vector.tensor_tensor(ysb[:, nsl], yp,
                                                bpb[:, nsl], op=ALU.add)
                        eng = nc.sync if n == 0 else nc.scalar
                        eng.dma_start(y_d.ap()[t0:t0 + 128, nsl],
                                      ysb[:, nsl])
                return emit

            def norm_proj_units(ic):
                us = [norm_unit(ic, p, h) for p in range(NP) for h in (0, 1)]
                us += [proj_unit(ic, tsub) for tsub in range(4)]
                return us

            # ---------------- attention pairs -----------------------------
            pairs = [(0, slice(0, 64)), (1, slice(64, 128))]

            class Filler:
                def __init__(self):
                    self.units = []
                    self.credit = 0.0
                    self.quota = 0.0

                def add(self, us, groups_left):
                    self.units.extend(us)
                    self.quota = len(self.units) / max(groups_left, 1)

                def step(self):
                    self.credit += self.quota
                    while self.units and self.credit >= 1.0:
                        self.units.pop(0)()
                        self.credit -= 1.0

                def flush(self):
                    for u in self.units:
                        u()
                    self.units = []

            def emit_scores(ic, p, g):
                isl = slice(512 * ic, 512 * (ic + 1))
                kind = g[0]
                if kind in ("full", "d0"):
                    jt = g[1] if kind == "full" else 4 * ic
                    sg = psgp.tile([128, 2, 512], f32, tag="sg",
                                   name=f"sg{ic}_{p}_{kind}{jt}")
                    for h, dsl in pairs:
                        nc.tensor.matmul(
                            sg[:, h, :],
                            kT[p][dsl, 128 * jt:128 * (jt + 1)],
                            qT[p][dsl, isl], start=True, stop=True)
                    return (sg, 512)
                if kind == "d1":
                    jt = 4 * ic + 1
                    sg = psgp.tile([128, 2, 512], f32, tag="sg",
                                   name=f"sgd1_{ic}_{p}")
                    for h, dsl in pairs:
                        nc.tensor.matmul(
                            sg[:, h, 0:384],
                            kT[p][dsl, 128 * jt:128 * (jt + 1)],
                            qT[p][dsl, 512 * ic + 128:512 * ic + 512],
                            start=True, stop=True)
                    return (sg, 384)
                # d23: two decreasing blocks packed per head
                sg = psgp.tile([128, 2, 512], f32, tag="sg",
                               name=f"sgd23_{ic}_{p}")
                for h, dsl in pairs:
                    nc.tensor.matmul(
                        sg[:, h, 0:256],
                        kT[p][dsl, 128 * (4 * ic + 2):128 * (4 * ic + 3)],
                        qT[p][dsl, 512 * ic + 256:512 * ic + 512],
                        start=True, stop=False)
                    nc.tensor.matmul(
                        sg[:, h, 256:384],
                        kT[p][dsl, 128 * (4 * ic + 3):128 * (4 * ic + 4)],
                        qT[p][dsl, 512 * ic + 384:512 * ic + 512],
                        start=False, stop=True)
                return (sg, 384)

            def emit_exp_av(ic, p, g, sg_w, avt, first):
                msk = late["msk"]
                kind = g[0]
                sg, w = sg_w
                et = esp.tile([128, 2, w], bf16, tag="et",
                              name=f"et{ic}_{p}_{kind}")
                nc.scalar.activation(et[:], sg[:, :, 0:w], AF.Exp,
                                     scale=SCALE)
                if kind in ("d0", "d1", "d23"):
                    nc.vector.tensor_tensor(et[:, :, 0:128],
                                            et[:, :, 0:128], msk[:],
                                            op=ALU.mult)
                if kind == "d23":
                    nc.vector.tensor_tensor(et[:, :, 256:384],
                                            et[:, :, 256:384], msk[:],
                                            op=ALU.mult)
                for h, dsl in pairs:
                    hl = 2 * p + h
                    if kind == "full":
                        nc.tensor.matmul(avt[:, h, :], v[g[1]][:, hl, :],
                                         et[:, h, :],
                                         start=first, stop=False)
                    elif kind == "d0":
                        nc.tensor.matmul(avt[:, h, :],
                                         v[4 * ic][:, hl, :], et[:, h, :],
                                         start=first, stop=False)
                    elif kind == "d1":
                        nc.tensor.matmul(avt[:, h, 128:512],
                                         v[4 * ic + 1][:, hl, :],
                                         et[:, h, :],
                                         start=False, stop=False)
                    else:
                        nc.tensor.matmul(avt[:, h, 256:512],
                                         v[4 * ic + 2][:, hl, :],
                                         et[:, h, 0:256],
                                         start=False, stop=False)
                        nc.tensor.matmul(avt[:, h, 384:512],
                                         v[4 * ic + 3][:, hl, :],
                                         et[:, h, 256:384],
                                         start=False, stop=True)

            def emit_lnexp(ic, rows=slice(0, 65)):
                # rr = exp(-ln S): batched 1/S over the S rows of a chunk
                lnS = snp.tile([65, 2, 512], f32, tag="lnS", bufs=2,
                               name=f"lnS{ic}_{rows.start}")
                n = rows.stop - rows.start
                nc.scalar.activation(lnS[0:n], S_all[rows, ic], AF.Ln)
                nc.scalar.activation(rr_all[rows, ic], lnS[0:n],
                                     AF.Exp, scale=-1.0)

            def emit_attn_pair(ic, p, fill, inline_norm=False):
                isl = slice(512 * ic, 512 * (ic + 1))
                avt = pavp.tile([65, 2, 512], f32, tag="av",
                                name=f"av{ic}_{p}")
                glist = [("full", jt) for jt in range(4 * ic)]
                glist += [("d0",), ("d1",), ("d23",)]
                sg_w = emit_scores(ic, p, glist[0])
                for i, g in enumerate(glist):
                    cur = sg_w
                    if i + 1 < len(glist):
                        sg_w = emit_scores(ic, p, glist[i + 1])
                    fill.step()
                    emit_exp_av(ic, p, g, cur, avt, first=(i == 0))
                # evacuate: unnormalized outT + S rows
                for h, dsl in pairs:
                    nc.vector.tensor_copy(outT[p][dsl, isl],
                                          avt[0:64, h, :])
                nc.vector.tensor_copy(S_all[32 * p:32 * p + 1, ic],
                                      avt[64:65, :, :])
                if inline_norm:
                    emit_lnexp(ic, rows=slice(32 * p, 32 * p + 1))
                    for h in (0, 1):
                        norm_unit(ic, p, h)()

            # ---------------- main schedule -------------------------------
            us0 = qkv_units(0)
            us0[0]()
            gate_xdma(1, (nc.gpsimd, nc.sync))()
            for u in us0[1:4]:
                u()
            gate_xdma(2, (nc.scalar, nc.gpsimd))()
            for u in us0[4:7]:
                u()
            gate_xdma(3, (nc.sync, nc.scalar))()
            for u in us0[7:]:
                u()
            emit_late_consts()
            fill = Filler()

            fill.add(qkv_units(1), 9)
            for p in range(NP):
                emit_attn_pair(0, p, fill)
            emit_lnexp(0)

            fill.add(qkv_units(2), 21)
            for p in range(NP):
                emit_attn_pair(1, p, fill)
            emit_lnexp(1)

            fill.add(qkv_units(3) + norm_proj_units(0), 33)
            for p in range(NP):
                emit_attn_pair(2, p, fill)
            emit_lnexp(2)

            fill.add(norm_proj_units(1) + norm_proj_units(2), 45)
            for p in range(NP):
                emit_attn_pair(3, p, fill, inline_norm=True)
            fill.flush()
            for tsub in range(4):
                proj_unit(3, tsub, pool=psgp if tsub % 2 else None)()

    _bacc_mod.get_activation_tables = _pinned_gat
    try:
        nc.compile()
    finally:
        _bacc_mod.get_activation_tables = _orig_gat
    return nc


def make_in_maps(x, w_attn, b_attn, w_proj, b_proj, T=T_FULL):
    import ml_dtypes
    bf = ml_dtypes.bfloat16
    x = np.asarray(x, np.float32)
    w_attn = np.asarray(w_attn, np.float32)
    b_attn = np.asarray(b_attn, np.float32)
    w_proj = np.asarray(w_proj, np.float32)
    b_proj = np.asarray(b_proj, np.float32)
    B = x.shape[0]

    ones = np.ones((128, 128), bf)
    # tril mask for the leading 128-column diagonal sub-block of each
    # stripe matmul (two identical copies, one per head)
    mask = np.broadcast_to(
        (np.arange(128)[:, None, None] <= np.arange(128)[None, None, :]),
        (128, 2, 128)).astype(np.float32)

    in_maps = []
    for c in range(N_CORES):
        b, g = (c // 2) % B, c % 2
        q0, k0, v0 = 384 * g, 768 + 384 * g, 1536 + 384 * g
        wqk = np.concatenate(
            [w_attn[:, q0:q0 + 384], w_attn[:, k0:k0 + 384]], axis=1)
        bqk = np.concatenate(
            [b_attn[q0:q0 + 384], b_attn[k0:k0 + 384]])
        in_maps.append({
            "xT": np.ascontiguousarray(x[b].T).astype(bf),
            "wqk": np.ascontiguousarray(wqk).astype(bf),
            "wv": np.ascontiguousarray(w_attn[:, v0:v0 + 384]).astype(bf),
            "wp": np.ascontiguousarray(w_proj[384 * g:384 * (g + 1), :]).astype(bf),
            "bqk": np.ascontiguousarray(bqk.reshape(6, 128).T),
            "bv": np.ascontiguousarray(b_attn[v0:v0 + 384].reshape(1, 384)).astype(bf),
            "bp": np.ascontiguousarray(
                (b_proj if g == 0 else np.zeros_like(b_proj)).reshape(1, -1)).astype(bf),
            "ones": ones,
            "mask": np.ascontiguousarray(mask).astype(bf),
        })
    return in_maps


def kernel(x, w_attn, b_attn, w_proj, b_proj):
    global LAST_RESULT
    if "nc" not in _NC_CACHE:
        _NC_CACHE["nc"] = build_nc(T_FULL)
    nc = _NC_CACHE["nc"]
    in_maps = make_in_maps(x, w_attn, b_attn, w_proj, b_proj)
    res = bass_utils.run_bass_kernel_spmd(
        nc, in_maps, core_ids=list(range(N_CORES)), trace=TRACE)
    LAST_RESULT = res
    B, T, C = np.asarray(x).shape
    y = np.empty((B, T, C), np.float32)
    for b in range(B):
        y[b] = res.results[2 * b]["y"] + res.results[2 * b + 1]["y"]
    return y
